# revision 11
# baseline (speedup 1.0000x reference)
"""GCN encoder (relu(A@x@W1+b1) -> A@h@{Wmu,Wlv}+{bmu,blv}) on 8 Trainium2
NeuronCores via Bass (axon-tunneled).

Sharding: nodes split contiguously across 8 cores (6272 padded nodes each,
49 tiles of 128).  Per core: local x@W1 (PE-transpose + bf16 matmul),
AllGather of scaled features, SpMM via indirect-DMA row gathers + on-device
one-hot selection matrices contracted on the PE array, AllGather again,
second SpMM, then the two output matmuls fused as one [128h]x[128h,128f].

Self-contained: needs numpy + concourse (on PYTHONPATH) + jax (axon).
"""
import os
import numpy as np

N = 50000
E = 800000
IN_C, HID, LAT = 256, 128, 64
N_CORES = 8
NP_CORE = 6272              # padded nodes per core (49 tiles)
NPAD = N_CORES * NP_CORE    # 50176
NT = NP_CORE // 128         # 49 tiles per core
CH = 17                     # gather chunks (128 edges) per tile (self-loops excluded)
P = 128

_RUNNER = None
_NC = None
XQ = [0, 13, 25, 37, 49]            # tile quarters for progressive allgather


# -------------------------------------------------------- landmark prepass --
def _landmarks():
    """Simulate per-engine instruction streams; return exact semaphore
    landmark tables.  Must mirror the emission order in _build_program."""
    L = {}
    # --- PE stream ---
    p = 0
    L["peA_mm"] = {}
    for j in range(NT):
        p += 2; L["peA_mm"][j] = p          # after both xw matmuls
    L["pe_c1"] = {}
    for j in range(NT):
        p += CH + 1; L["pe_c1"][j] = p      # identity(self) + CH chunks
    L["pe_mm2"] = {}; L["pe_O"] = {}
    for j in range(NT):
        p += CH + 1; L["pe_mm2"][j] = p
        if j >= 1:
            p += 1; L["pe_O"][j - 1] = p
    p += 1; L["pe_O"][NT - 1] = p
    L["pe_total"] = p

    # --- DVE stream ---
    v = 0
    L["dveA_s"] = {}
    for j in range(NT):
        v += 1; L["dveA_s"][j] = v
    L["vS1"] = {}; L["vE1"] = {}

    def sim_s1(j):
        nonlocal v
        v += CH; L["vS1"][j] = v

    def sim_e1(j):
        nonlocal v
        v += 2; L["vE1"][j] = v

    sim_s1(0)
    for j in range(1, NT):
        sim_s1(j); sim_e1(j - 1)
    sim_e1(NT - 1)

    L["vS2"] = {}; L["vC2"] = {}; L["vE2"] = {}

    def sim_s2(j):
        nonlocal v
        v += CH; L["vS2"][j] = v

    def sim_c2(j):
        nonlocal v
        v += 1; L["vC2"][j] = v

    def sim_e2(j):
        nonlocal v
        v += 2; L["vE2"][j] = v

    sim_s2(0)
    sim_s2(1); sim_c2(0)
    for j in range(2, NT):
        sim_s2(j); sim_c2(j - 1); sim_e2(j - 2)
    sim_c2(NT - 1); sim_e2(NT - 2); sim_e2(NT - 1)
    L["v_total"] = v
    return L


# ----------------------------------------------------------------- device --
def _build_program():
    import concourse.bass as bass
    import concourse.mybir as mybir
    from concourse.bass import IndirectOffsetOnAxis
    from contextlib import ExitStack

    f32, bf16, i32 = mybir.dt.float32, mybir.dt.bfloat16, mybir.dt.int32
    LM = _landmarks()

    nc = bass.Bass()

    xt_in = nc.dram_tensor("xt", [IN_C, NP_CORE], bf16, kind="ExternalInput")
    w1_in = nc.dram_tensor("w1", [IN_C, HID], bf16, kind="ExternalInput")
    wm_in = nc.dram_tensor("wmulv", [HID, 2 * LAT], bf16, kind="ExternalInput")
    iota_in = nc.dram_tensor("iota", [P, P], f32, kind="ExternalInput")
    b1f_in = nc.dram_tensor("b1f", [P, HID], f32, kind="ExternalInput")
    bmf_in = nc.dram_tensor("bmf", [P, 2 * LAT], f32, kind="ExternalInput")
    dinv_in = nc.dram_tensor("dinv", [P, NT], f32, kind="ExternalInput")
    rows_in = nc.dram_tensor("rows", [P, NT * CH], i32, kind="ExternalInput")
    colrel_in = nc.dram_tensor("colrel", [P, NT * CH], f32, kind="ExternalInput")
    out_ext = nc.dram_tensor("out", [NP_CORE, 2 * LAT], f32, kind="ExternalOutput")

    xws_local = nc.dram_tensor("xws_local", [NP_CORE, HID], bf16)
    xws_full = nc.dram_tensor("xws_full", [NPAD, HID], bf16, addr_space="Shared")
    hs_local = nc.dram_tensor("hs_local", [NP_CORE, HID], bf16)
    hs_full = nc.dram_tensor("hs_full", [NPAD, HID], bf16, addr_space="Shared")

    es = ExitStack()
    sb = lambda name, shape, dt: es.enter_context(nc.sbuf_tensor(name, shape, dt))
    xt_sb = sb("xt_sb", [P, 2 * NP_CORE], bf16)
    w1_sb = sb("w1_sb", [P, 2 * HID], bf16)
    wm_sb = sb("wm_sb", [P, 2 * LAT], bf16)
    iota_sb = sb("iota_sb", [P, P], f32)
    b1f_sb = sb("b1f_sb", [P, HID], f32)
    bmf_sb = sb("bmf_sb", [P, 2 * LAT], f32)
    dinv_sb = sb("dinv_sb", [P, NT], f32)
    rows_sb = sb("rows_sb", [P, NT * CH], i32)
    colrel_sb = sb("colrel_sb", [P, NT * CH], f32)
    ident_sb = sb("ident_sb", [P, P], bf16)
    xws_all = sb("xws_all", [P, NT * HID], bf16)
    hs_all = sb("hs_all", [P, NT * HID], bf16)
    out_all = sb("out_all", [P, NT * 2 * LAT], f32)
    msg_sb = [sb(f"msg_sb{i}", [P, CH * P], bf16) for i in range(3)]
    s_sb = [sb(f"s_sb{i}", [P, CH * P], bf16) for i in range(3)]
    st_sb = [sb(f"st_sb{i}", [P, P], bf16) for i in range(2)]
    tmp1_sb = [sb(f"tmp1_sb{i}", [P, P], f32) for i in range(2)]
    tmp2_sb = [sb(f"tmp2_sb{i}", [P, P], f32) for i in range(2)]
    otmp_sb = [sb(f"otmp_sb{i}", [P, 2 * LAT], f32) for i in range(2)]

    psum = lambda name, dt: es.enter_context(nc.psum_tensor(name, [P, P], dt))
    psxw = [psum(f"psxw_{i}", f32) for i in range(2)]
    psc1 = [psum(f"psc1_{i}", f32) for i in range(2)]
    pss2 = psxw    # conv2 reuses f32 banks (phase A / conv1 drained by then)
    psout = psc1

    sem = lambda name: es.enter_context(nc.semaphore(name))
    d_sem = sem("d")
    g1_sem = sem("g1")
    g2_sem = sem("g2")
    p_sem = sem("p")
    v_sem = sem("v")
    a_sem = sem("a")
    m_sem = sem("m")
    c_sem = sem("c")

    D0 = 16 * 9  # 8 gpsimd loads + x load (sync queue, same sem)

    with nc.Block() as block:

        @block.sync
        def _(sync):
            sync.dma_start(
                out=xt_sb[:].rearrange("p (c n) -> p c n", c=2),
                in_=xt_in.rearrange("(c p) n -> p c n", p=P),
            ).then_inc(d_sem, 16)

        @block.gpsimd
        def _(gps):
            gps.dma_start(
                out=w1_sb[:].rearrange("p (c h) -> p c h", c=2),
                in_=w1_in.rearrange("(c p) h -> p c h", p=P),
            ).then_inc(d_sem, 16)
            gps.dma_start(out=wm_sb[:], in_=wm_in[:]).then_inc(d_sem, 16)
            gps.dma_start(out=iota_sb[:], in_=iota_in[:]).then_inc(d_sem, 16)
            gps.dma_start(out=b1f_sb[:], in_=b1f_in[:]).then_inc(d_sem, 16)
            gps.dma_start(out=bmf_sb[:], in_=bmf_in[:]).then_inc(d_sem, 16)
            gps.dma_start(out=dinv_sb[:], in_=dinv_in[:]).then_inc(d_sem, 16)
            gps.dma_start(out=rows_sb[:], in_=rows_in[:]).then_inc(d_sem, 16)
            gps.dma_start(out=colrel_sb[:], in_=colrel_in[:]).then_inc(d_sem, 16)
            gps.memset(ident_sb[:], 0.0)
            gps.affine_select(
                out=ident_sb[:], in_=ident_sb[:],
                compare_op=mybir.AluOpType.not_equal, fill=1.0,
                base=0, pattern=[[-1, P]], channel_multiplier=1,
            ).then_inc(m_sem, 1)

            # xws quarter stores + progressive allgathers
            QR = [N_CORES * P * t for t in XQ]
            for q in range(4):
                t0q, t1q = XQ[q], XQ[q + 1]
                gps.wait_ge(v_sem, LM["dveA_s"][t1q - 1])
                gps.dma_start(
                    out=xws_local[t0q * P:t1q * P].rearrange("(j p) h -> p j h", p=P),
                    in_=xws_all[:, t0q * HID:t1q * HID].rearrange("p (j h) -> p j h", j=t1q - t0q),
                ).then_inc(d_sem, 16)
                gps.wait_ge(d_sem, D0 + 16 * (q + 1))
                gps.collective_compute(
                    "AllGather", mybir.AluOpType.bypass,
                    ins=[xws_local[t0q * P:t1q * P]],
                    outs=[xws_full[QR[q]:QR[q + 1]]],
                    replica_groups=[list(range(N_CORES))],
                ).then_inc(c_sem, 1)
            gps.wait_ge(c_sem, 4)

            for j in range(NT):
                if j >= 3:
                    gps.wait_ge(p_sem, LM["pe_c1"][j - 3])
                for ch in range(CH):
                    q = j * CH + ch
                    gps.indirect_dma_start(
                        out=msg_sb[j % 3][:, ch * P:(ch + 1) * P],
                        out_offset=None,
                        in_=xws_full[:],
                        in_offset=IndirectOffsetOnAxis(ap=rows_sb[:, q:q + 1], axis=0),
                    ).then_inc(g1_sem, 16)

            # hs quarter stores + progressive allgathers
            for q in range(4):
                t0q, t1q = XQ[q], XQ[q + 1]
                gps.wait_ge(a_sem, t1q)
                gps.dma_start(
                    out=hs_local[t0q * P:t1q * P].rearrange("(j p) h -> p j h", p=P),
                    in_=hs_all[:, t0q * HID:t1q * HID].rearrange("p (j h) -> p j h", j=t1q - t0q),
                ).then_inc(d_sem, 16)
                gps.wait_ge(d_sem, D0 + 64 + 16 * (q + 1))
                gps.collective_compute(
                    "AllGather", mybir.AluOpType.bypass,
                    ins=[hs_local[t0q * P:t1q * P]],
                    outs=[hs_full[QR[q]:QR[q + 1]]],
                    replica_groups=[list(range(N_CORES))],
                ).then_inc(c_sem, 1)
            gps.wait_ge(c_sem, 8)

            for j in range(NT):
                if j >= 3:
                    gps.wait_ge(p_sem, LM["pe_mm2"][j - 3])
                for ch in range(CH):
                    q = j * CH + ch
                    gps.indirect_dma_start(
                        out=msg_sb[j % 3][:, ch * P:(ch + 1) * P],
                        out_offset=None,
                        in_=hs_full[:],
                        in_offset=IndirectOffsetOnAxis(ap=rows_sb[:, q:q + 1], axis=0),
                    ).then_inc(g2_sem, 16)

            gps.wait_ge(v_sem, LM["vE2"][NT - 1])
            gps.dma_start(
                out=out_ext.rearrange("(j p) f -> p j f", p=P),
                in_=out_all[:].rearrange("p (j f) -> p j f", j=NT),
            ).then_inc(d_sem, 16)
            gps.wait_ge(d_sem, D0 + 144)

        @block.tensor
        def _(pe):
            pe.wait_ge(d_sem, D0)
            pe.wait_ge(m_sem, 1)
            for j in range(NT):
                if j >= 2:
                    pe.wait_ge(v_sem, LM["dveA_s"][j - 2])
                nc.tensor.matmul(
                    out=psxw[j % 2][:],
                    lhsT=xt_sb[:, j * P:(j + 1) * P],
                    rhs=w1_sb[:, 0:HID], start=True, stop=False,
                ).then_inc(p_sem, 1)
                nc.tensor.matmul(
                    out=psxw[j % 2][:],
                    lhsT=xt_sb[:, NP_CORE + j * P:NP_CORE + (j + 1) * P],
                    rhs=w1_sb[:, HID:2 * HID], start=False, stop=True,
                ).then_inc(p_sem, 1)

            for j in range(NT):
                pe.wait_ge(g1_sem, 16 * CH * (j + 1))
                pe.wait_ge(v_sem, LM["vS1"][j])
                nc.tensor.matmul(
                    out=psc1[j % 2][:], lhsT=ident_sb[:],
                    rhs=xws_all[:, j * HID:(j + 1) * HID],
                    start=True, stop=False,
                ).then_inc(p_sem, 1)
                for ch in range(CH):
                    nc.tensor.matmul(
                        out=psc1[j % 2][:],
                        lhsT=s_sb[j % 3][:, ch * P:(ch + 1) * P],
                        rhs=msg_sb[j % 3][:, ch * P:(ch + 1) * P],
                        start=False, stop=(ch == CH - 1),
                    ).then_inc(p_sem, 1)

            def mm_out(j):
                pe.wait_ge(v_sem, LM["vC2"][j])
                nc.tensor.matmul(
                    out=psout[j % 2][:, 0:2 * LAT],
                    lhsT=st_sb[j % 2][:], rhs=wm_sb[:],
                    start=True, stop=True,
                ).then_inc(p_sem, 1)

            for j in range(NT):
                pe.wait_ge(g2_sem, 16 * CH * (j + 1))
                pe.wait_ge(v_sem, LM["vS2"][j])
                nc.tensor.matmul(
                    out=pss2[j % 2][:],
                    lhsT=hs_all[:, j * HID:(j + 1) * HID],
                    rhs=ident_sb[:],
                    start=True, stop=False,
                ).then_inc(p_sem, 1)
                for ch in range(CH):
                    nc.tensor.matmul(
                        out=pss2[j % 2][:],
                        lhsT=msg_sb[j % 3][:, ch * P:(ch + 1) * P],
                        rhs=s_sb[j % 3][:, ch * P:(ch + 1) * P],
                        start=False, stop=(ch == CH - 1),
                    ).then_inc(p_sem, 1)
                if j >= 1:
                    mm_out(j - 1)
            mm_out(NT - 1)

        @block.vector
        def _(dve):
            for j in range(NT):
                dve.wait_ge(p_sem, LM["peA_mm"][j])
                nc.vector.tensor_scalar(
                    out=xws_all[:, j * HID:(j + 1) * HID], in0=psxw[j % 2][:],
                    scalar1=dinv_sb[:, j:j + 1], scalar2=None,
                    op0=mybir.AluOpType.mult,
                ).then_inc(v_sem, 1)

            def s_build(j, war_lm):
                if war_lm is not None:
                    dve.wait_ge(p_sem, war_lm)
                for ch in range(CH):
                    q = j * CH + ch
                    nc.vector.tensor_tensor(
                        out=s_sb[j % 3][:, ch * P:(ch + 1) * P],
                        in0=colrel_sb[:, q:q + 1].to_broadcast([P, P]),
                        in1=iota_sb[:],
                        op=mybir.AluOpType.is_equal,
                    ).then_inc(v_sem, 1)

            def epi_c1(j):
                dve.wait_ge(p_sem, LM["pe_c1"][j])
                nc.vector.tensor_scalar(
                    out=tmp1_sb[j % 2][:], in0=psc1[j % 2][:],
                    scalar1=dinv_sb[:, j:j + 1], scalar2=None,
                    op0=mybir.AluOpType.mult,
                ).then_inc(v_sem, 1)
                if j >= 2:
                    dve.wait_ge(a_sem, j - 1)
                nc.vector.tensor_tensor(
                    out=tmp2_sb[j % 2][:], in0=tmp1_sb[j % 2][:], in1=b1f_sb[:],
                    op=mybir.AluOpType.add,
                ).then_inc(v_sem, 1)

            s_build(0, None)
            for j in range(1, NT):
                s_build(j, LM["pe_c1"][j - 3] if j >= 3 else None)
                epi_c1(j - 1)
            epi_c1(NT - 1)

            def copy_c2(j):
                dve.wait_ge(p_sem, LM["pe_mm2"][j])
                nc.vector.tensor_copy(out=st_sb[j % 2][:], in_=pss2[j % 2][:]).then_inc(v_sem, 1)

            def epi_c2(j):
                dve.wait_ge(p_sem, LM["pe_O"][j])
                nc.vector.tensor_scalar(
                    out=otmp_sb[j % 2][:], in0=psout[j % 2][:, 0:2 * LAT],
                    scalar1=dinv_sb[:, j:j + 1], scalar2=None,
                    op0=mybir.AluOpType.mult,
                ).then_inc(v_sem, 1)
                nc.vector.tensor_tensor(
                    out=out_all[:, j * 2 * LAT:(j + 1) * 2 * LAT],
                    in0=otmp_sb[j % 2][:], in1=bmf_sb[:],
                    op=mybir.AluOpType.add,
                ).then_inc(v_sem, 1)

            # conv2 stream (mirrors _landmarks sim exactly)
            s_build2 = lambda j, war: s_build(j, war)
            s_build2(0, None)
            s_build2(1, None)
            copy_c2(0)
            for j in range(2, NT):
                s_build2(j, LM["pe_mm2"][j - 3] if j >= 3 else None)
                copy_c2(j - 1)
                epi_c2(j - 2)
            copy_c2(NT - 1)
            epi_c2(NT - 2)
            epi_c2(NT - 1)

        @block.scalar
        def _(act):
            for j in range(NT):
                act.wait_ge(v_sem, LM["vE1"][j])
                nc.scalar.activation(
                    out=hs_all[:, j * HID:(j + 1) * HID],
                    in_=tmp2_sb[j % 2][:],
                    func=mybir.ActivationFunctionType.Relu,
                    scale=dinv_sb[:, j:j + 1],
                ).then_inc(a_sem, 1)

    es.close()
    return nc


# ------------------------------------------------------------------- host --
def _preprocess(x, edge_index, W1, b1, Wmu, bmu, Wlv, blv):
    import ml_dtypes
    bf16 = ml_dtypes.bfloat16

    ei = np.asarray(edge_index)
    srcs = ei[0].astype(np.int64)
    dst = ei[1].astype(np.int64)

    # degree includes the self-loop; self-loop contribution is added on-device
    deg = (np.bincount(dst, minlength=N) + 1).astype(np.float32)
    dinv = np.zeros(NPAD, dtype=np.float32)
    dinv[:N] = 1.0 / np.sqrt(deg)

    order = np.argsort(dst, kind="stable")
    rs = srcs[order].astype(np.int32)
    cs = dst[order]

    tile_of = cs // P
    n_tiles = NPAD // P
    tile_cnt = np.bincount(tile_of, minlength=n_tiles)
    if tile_cnt.max() > CH * P:
        raise OverflowError("tile edge count exceeds static chunk budget")
    tile_start = np.zeros(n_tiles + 1, dtype=np.int64)
    np.cumsum(tile_cnt, out=tile_start[1:])
    rank = np.arange(len(cs)) - tile_start[tile_of]
    pos = tile_of * (CH * P) + rank

    # remap row ids to the quartered xws_full/hs_full layout:
    # quarter q holds rows {core k, local tile in [XQ[q], XQ[q+1])} contiguously
    v_all = np.arange(NPAD, dtype=np.int64)
    k_of = v_all // NP_CORE
    r_of = v_all % NP_CORE
    t_of = r_of // P
    q_of = np.searchsorted(np.asarray(XQ[1:]), t_of, side="right")
    nq = np.diff(np.asarray(XQ)) * P                      # rows per core per quarter
    QR = np.concatenate([[0], np.cumsum(N_CORES * nq)])   # quarter bases
    remap = (QR[q_of] + k_of * nq[q_of] + (r_of - np.asarray(XQ)[q_of] * P)).astype(np.int32)

    rows_pad = np.zeros(n_tiles * CH * P, dtype=np.int32)
    colr_pad = np.full(n_tiles * CH * P, 999.0, dtype=np.float32)
    rows_pad[pos] = remap[rs]
    colr_pad[pos] = (cs - tile_of * P).astype(np.float32)
    rows_pad = rows_pad.reshape(N_CORES, NT * CH, P)
    colr_pad = colr_pad.reshape(N_CORES, NT * CH, P)

    xb = np.asarray(x, dtype=np.float32).astype(bf16)
    x_pad = np.zeros((NPAD, IN_C), dtype=bf16)
    x_pad[:N] = xb
    xT = np.ascontiguousarray(x_pad.T)                        # [IN_C, NPAD]
    xt_cat = np.ascontiguousarray(
        xT.reshape(IN_C, N_CORES, NP_CORE).transpose(1, 0, 2)
    ).reshape(N_CORES * IN_C, NP_CORE)

    w1 = np.asarray(W1, dtype=np.float32).astype(bf16)
    wm = np.concatenate([np.asarray(Wmu), np.asarray(Wlv)], axis=1).astype(np.float32).astype(bf16)
    iota = np.tile(np.arange(P, dtype=np.float32), (P, 1))
    b1f = np.tile(np.asarray(b1, dtype=np.float32)[None, :], (P, 1))
    bmf = np.tile(np.concatenate([np.asarray(bmu), np.asarray(blv)]).astype(np.float32)[None, :], (P, 1))

    dinv_t = dinv.reshape(N_CORES, NT, P).transpose(0, 2, 1)
    rows_t = np.ascontiguousarray(rows_pad.transpose(0, 2, 1))
    colr_t = np.ascontiguousarray(colr_pad.transpose(0, 2, 1))

    cat = {
        "xt": xt_cat,
        "w1": np.tile(w1, (N_CORES, 1)),
        "wmulv": np.tile(wm, (N_CORES, 1)),
        "iota": np.tile(iota, (N_CORES, 1)),
        "b1f": np.tile(b1f, (N_CORES, 1)),
        "bmf": np.tile(bmf, (N_CORES, 1)),
        "dinv": np.ascontiguousarray(dinv_t.reshape(N_CORES * P, NT)),
        "rows": np.ascontiguousarray(rows_t.reshape(N_CORES * P, NT * CH)),
        "colrel": np.ascontiguousarray(colr_t.reshape(N_CORES * P, NT * CH)),
    }
    return cat


def make_in_maps(cat):
    """Split concat inputs back into per-core maps (for test/profiling)."""
    maps = []
    for c in range(N_CORES):
        m = {}
        for k, v in cat.items():
            per = v.shape[0] // N_CORES
            m[k] = np.ascontiguousarray(v[c * per:(c + 1) * per])
        maps.append(m)
    return maps


def _get_runner():
    global _RUNNER, _NC
    if _RUNNER is None:
        import jax
        from jax.sharding import Mesh, PartitionSpec
        from jax.experimental.shard_map import shard_map
        import concourse.mybir as mybir
        from concourse.bass2jax import (
            _bass_exec_p, install_neuronx_cc_hook, partition_id_tensor,
        )

        install_neuronx_cc_hook()
        nc = _build_program()
        _NC = nc

        partition_name = nc.partition_id_tensor.name if nc.partition_id_tensor else None
        in_names, out_names, out_avals = [], [], []
        for alloc in nc.m.functions[0].allocations:
            if not isinstance(alloc, mybir.MemoryLocationSet):
                continue
            name = alloc.memorylocations[0].name
            if alloc.kind == "ExternalInput":
                if name != partition_name:
                    in_names.append(name)
            elif alloc.kind == "ExternalOutput":
                out_names.append(name)
                out_avals.append(jax.core.ShapedArray(
                    tuple(alloc.tensor_shape), mybir.dt.np(alloc.dtype)))

        n_params, n_outs = len(in_names), len(out_names)
        all_in_names = in_names + out_names + ([partition_name] if partition_name else [])
        donate = tuple(range(n_params, n_params + n_outs))

        def _body(*args):
            operands = list(args)
            if partition_name is not None:
                operands.append(partition_id_tensor())
            return tuple(_bass_exec_p.bind(
                *operands,
                out_avals=tuple(out_avals),
                in_names=tuple(all_in_names),
                out_names=tuple(out_names),
                lowering_input_output_aliases=(),
                sim_require_finite=False,
                sim_require_nnan=False,
                nc=nc,
            ))

        devices = jax.devices()[:N_CORES]
        mesh = Mesh(np.asarray(devices), ("core",))
        fn = jax.jit(
            shard_map(
                _body, mesh=mesh,
                in_specs=(PartitionSpec("core"),) * (n_params + n_outs),
                out_specs=(PartitionSpec("core"),) * n_outs,
                check_rep=False,
            ),
            donate_argnums=donate, keep_unused=True,
        )

        def call(cat):
            args = [cat[n] for n in in_names]
            zeros = [np.zeros((N_CORES * a.shape[0], *a.shape[1:]), a.dtype)
                     for a in out_avals]
            outs = fn(*args, *zeros)
            return {n: np.asarray(outs[i]) for i, n in enumerate(out_names)}

        _RUNNER = call
    return _RUNNER


def kernel(x, edge_index, W1, b1, Wmu, bmu, Wlv, blv):
    run = _get_runner()
    cat = _preprocess(x, edge_index, W1, b1, Wmu, bmu, Wlv, blv)
    out = run(cat)["out"]
    mu = out.reshape(NPAD, 2 * LAT)[:N, :LAT]
    logvar = out.reshape(NPAD, 2 * LAT)[:N, LAT:]
    return (np.ascontiguousarray(mu), np.ascontiguousarray(logvar))


if os.environ.get("GCN_NO_WARMUP") != "1" and __name__ != "__main__":
    try:
        _get_runner()
    except Exception:
        _RUNNER = None


# revision 12
# speedup vs baseline: 1.0377x; 1.0377x over previous
"""GCN encoder (relu(A@x@W1+b1) -> A@h@{Wmu,Wlv}+{bmu,blv}) on 8 Trainium2
NeuronCores via Bass (axon-tunneled).

Sharding: nodes split contiguously across 8 cores (6272 padded nodes each,
49 tiles of 128).  Per core: local x@W1 (PE-transpose + bf16 matmul),
AllGather of scaled features, SpMM via indirect-DMA row gathers + on-device
one-hot selection matrices contracted on the PE array, AllGather again,
second SpMM, then the two output matmuls fused as one [128h]x[128h,128f].

Self-contained: needs numpy + concourse (on PYTHONPATH) + jax (axon).
"""
import os
import numpy as np

N = 50000
E = 800000
IN_C, HID, LAT = 256, 128, 64
N_CORES = 8
NP_CORE = 6272              # padded nodes per core (49 tiles)
NPAD = N_CORES * NP_CORE    # 50176
NT = NP_CORE // 128         # 49 tiles per core
CH = 17                     # gather chunks (128 edges) per tile (self-loops excluded)
P = 128

_RUNNER = None
_NC = None
XQ = [0, 49]                        # single allgather (fixed ~20us per collective)


# -------------------------------------------------------- landmark prepass --
def _landmarks():
    """Simulate per-engine instruction streams; return exact semaphore
    landmark tables.  Must mirror the emission order in _build_program."""
    L = {}
    # --- PE stream ---
    p = 0
    L["peA_mm"] = {}
    for j in range(NT):
        p += 2; L["peA_mm"][j] = p          # after both xw matmuls
    L["pe_c1"] = {}
    for j in range(NT):
        p += CH + 1; L["pe_c1"][j] = p      # identity(self) + CH chunks
    L["pe_mm2"] = {}; L["pe_O"] = {}
    for j in range(NT):
        p += CH + 1; L["pe_mm2"][j] = p
        if j >= 1:
            p += 1; L["pe_O"][j - 1] = p
    p += 1; L["pe_O"][NT - 1] = p
    L["pe_total"] = p

    # --- DVE stream ---
    v = 0
    L["dveA_s"] = {}
    for j in range(NT):
        v += 1; L["dveA_s"][j] = v
    L["vS1"] = {}; L["vE1"] = {}

    def sim_s1(j):
        nonlocal v
        v += CH; L["vS1"][j] = v

    def sim_e1(j):
        nonlocal v
        v += 2; L["vE1"][j] = v

    sim_s1(0)
    for j in range(1, NT):
        sim_s1(j); sim_e1(j - 1)
    sim_e1(NT - 1)

    L["vS2"] = {}; L["vC2"] = {}; L["vE2"] = {}

    def sim_s2(j):
        nonlocal v
        v += CH; L["vS2"][j] = v

    def sim_c2(j):
        nonlocal v
        v += 1; L["vC2"][j] = v

    def sim_e2(j):
        nonlocal v
        v += 2; L["vE2"][j] = v

    sim_s2(0)
    sim_s2(1); sim_c2(0)
    for j in range(2, NT):
        sim_s2(j); sim_c2(j - 1); sim_e2(j - 2)
    sim_c2(NT - 1); sim_e2(NT - 2); sim_e2(NT - 1)
    L["v_total"] = v
    return L


# ----------------------------------------------------------------- device --
def _build_program():
    import concourse.bass as bass
    import concourse.mybir as mybir
    from concourse.bass import IndirectOffsetOnAxis
    from contextlib import ExitStack

    f32, bf16, i32 = mybir.dt.float32, mybir.dt.bfloat16, mybir.dt.int32
    LM = _landmarks()

    nc = bass.Bass()

    xt_in = nc.dram_tensor("xt", [IN_C, NP_CORE], bf16, kind="ExternalInput")
    w1_in = nc.dram_tensor("w1", [IN_C, HID], bf16, kind="ExternalInput")
    wm_in = nc.dram_tensor("wmulv", [HID, 2 * LAT], bf16, kind="ExternalInput")
    iota_in = nc.dram_tensor("iota", [P, P], f32, kind="ExternalInput")
    b1f_in = nc.dram_tensor("b1f", [P, HID], f32, kind="ExternalInput")
    bmf_in = nc.dram_tensor("bmf", [P, 2 * LAT], f32, kind="ExternalInput")
    dinv_in = nc.dram_tensor("dinv", [P, NT], f32, kind="ExternalInput")
    rows_in = nc.dram_tensor("rows", [P, NT * CH], i32, kind="ExternalInput")
    colrel_in = nc.dram_tensor("colrel", [P, NT * CH], f32, kind="ExternalInput")
    out_ext = nc.dram_tensor("out", [NP_CORE, 2 * LAT], f32, kind="ExternalOutput")

    xws_local = nc.dram_tensor("xws_local", [NP_CORE, HID], bf16)
    xws_full = nc.dram_tensor("xws_full", [NPAD, HID], bf16, addr_space="Shared")
    hs_local = nc.dram_tensor("hs_local", [NP_CORE, HID], bf16)
    hs_full = nc.dram_tensor("hs_full", [NPAD, HID], bf16, addr_space="Shared")

    es = ExitStack()
    sb = lambda name, shape, dt: es.enter_context(nc.sbuf_tensor(name, shape, dt))
    xt_sb = sb("xt_sb", [P, 2 * NP_CORE], bf16)
    w1_sb = sb("w1_sb", [P, 2 * HID], bf16)
    wm_sb = sb("wm_sb", [P, 2 * LAT], bf16)
    iota_sb = sb("iota_sb", [P, P], f32)
    b1f_sb = sb("b1f_sb", [P, HID], f32)
    bmf_sb = sb("bmf_sb", [P, 2 * LAT], f32)
    dinv_sb = sb("dinv_sb", [P, NT], f32)
    rows_sb = sb("rows_sb", [P, NT * CH], i32)
    colrel_sb = sb("colrel_sb", [P, NT * CH], f32)
    ident_sb = sb("ident_sb", [P, P], bf16)
    xws_all = sb("xws_all", [P, NT * HID], bf16)
    hs_all = sb("hs_all", [P, NT * HID], bf16)
    out_all = sb("out_all", [P, NT * 2 * LAT], f32)
    msg_sb = [sb(f"msg_sb{i}", [P, CH * P], bf16) for i in range(3)]
    s_sb = [sb(f"s_sb{i}", [P, CH * P], bf16) for i in range(3)]
    st_sb = [sb(f"st_sb{i}", [P, P], bf16) for i in range(2)]
    tmp1_sb = [sb(f"tmp1_sb{i}", [P, P], f32) for i in range(2)]
    tmp2_sb = [sb(f"tmp2_sb{i}", [P, P], f32) for i in range(2)]
    otmp_sb = [sb(f"otmp_sb{i}", [P, 2 * LAT], f32) for i in range(2)]

    psum = lambda name, dt: es.enter_context(nc.psum_tensor(name, [P, P], dt))
    psxw = [psum(f"psxw_{i}", f32) for i in range(2)]
    psc1 = [psum(f"psc1_{i}", f32) for i in range(2)]
    pss2 = psxw    # conv2 reuses f32 banks (phase A / conv1 drained by then)
    psout = psc1

    sem = lambda name: es.enter_context(nc.semaphore(name))
    d_sem = sem("d")
    g1_sem = sem("g1")
    g2_sem = sem("g2")
    p_sem = sem("p")
    v_sem = sem("v")
    a_sem = sem("a")
    m_sem = sem("m")
    c_sem = sem("c")

    D0 = 16 * 9  # 8 gpsimd loads + x load (sync queue, same sem)

    with nc.Block() as block:

        @block.sync
        def _(sync):
            sync.dma_start(
                out=xt_sb[:].rearrange("p (c n) -> p c n", c=2),
                in_=xt_in.rearrange("(c p) n -> p c n", p=P),
            ).then_inc(d_sem, 16)

        @block.gpsimd
        def _(gps):
            gps.dma_start(
                out=w1_sb[:].rearrange("p (c h) -> p c h", c=2),
                in_=w1_in.rearrange("(c p) h -> p c h", p=P),
            ).then_inc(d_sem, 16)
            gps.dma_start(out=wm_sb[:], in_=wm_in[:]).then_inc(d_sem, 16)
            gps.dma_start(out=iota_sb[:], in_=iota_in[:]).then_inc(d_sem, 16)
            gps.dma_start(out=b1f_sb[:], in_=b1f_in[:]).then_inc(d_sem, 16)
            gps.dma_start(out=bmf_sb[:], in_=bmf_in[:]).then_inc(d_sem, 16)
            gps.dma_start(out=dinv_sb[:], in_=dinv_in[:]).then_inc(d_sem, 16)
            gps.dma_start(out=rows_sb[:], in_=rows_in[:]).then_inc(d_sem, 16)
            gps.dma_start(out=colrel_sb[:], in_=colrel_in[:]).then_inc(d_sem, 16)
            gps.memset(ident_sb[:], 0.0)
            gps.affine_select(
                out=ident_sb[:], in_=ident_sb[:],
                compare_op=mybir.AluOpType.not_equal, fill=1.0,
                base=0, pattern=[[-1, P]], channel_multiplier=1,
            ).then_inc(m_sem, 1)

            # xws quarter stores + progressive allgathers
            QR = [N_CORES * P * t for t in XQ]
            for q in range(len(XQ) - 1):
                t0q, t1q = XQ[q], XQ[q + 1]
                gps.wait_ge(v_sem, LM["dveA_s"][t1q - 1])
                gps.dma_start(
                    out=xws_local[t0q * P:t1q * P].rearrange("(j p) h -> p j h", p=P),
                    in_=xws_all[:, t0q * HID:t1q * HID].rearrange("p (j h) -> p j h", j=t1q - t0q),
                ).then_inc(d_sem, 16)
                gps.wait_ge(d_sem, D0 + 16 * (q + 1))
                gps.collective_compute(
                    "AllGather", mybir.AluOpType.bypass,
                    ins=[xws_local[t0q * P:t1q * P]],
                    outs=[xws_full[QR[q]:QR[q + 1]]],
                    replica_groups=[list(range(N_CORES))],
                ).then_inc(c_sem, 1)
            gps.wait_ge(c_sem, len(XQ) - 1)

            for j in range(NT):
                if j >= 3:
                    gps.wait_ge(p_sem, LM["pe_c1"][j - 3])
                for ch in range(CH):
                    q = j * CH + ch
                    gps.indirect_dma_start(
                        out=msg_sb[j % 3][:, ch * P:(ch + 1) * P],
                        out_offset=None,
                        in_=xws_full[:],
                        in_offset=IndirectOffsetOnAxis(ap=rows_sb[:, q:q + 1], axis=0),
                    ).then_inc(g1_sem, 16)

            # hs quarter stores + progressive allgathers
            for q in range(len(XQ) - 1):
                t0q, t1q = XQ[q], XQ[q + 1]
                gps.wait_ge(a_sem, t1q)
                gps.dma_start(
                    out=hs_local[t0q * P:t1q * P].rearrange("(j p) h -> p j h", p=P),
                    in_=hs_all[:, t0q * HID:t1q * HID].rearrange("p (j h) -> p j h", j=t1q - t0q),
                ).then_inc(d_sem, 16)
                gps.wait_ge(d_sem, D0 + 16 * (len(XQ) - 1) + 16 * (q + 1))
                gps.collective_compute(
                    "AllGather", mybir.AluOpType.bypass,
                    ins=[hs_local[t0q * P:t1q * P]],
                    outs=[hs_full[QR[q]:QR[q + 1]]],
                    replica_groups=[list(range(N_CORES))],
                ).then_inc(c_sem, 1)
            gps.wait_ge(c_sem, 2 * (len(XQ) - 1))

            for j in range(NT):
                if j >= 3:
                    gps.wait_ge(p_sem, LM["pe_mm2"][j - 3])
                for ch in range(CH):
                    q = j * CH + ch
                    gps.indirect_dma_start(
                        out=msg_sb[j % 3][:, ch * P:(ch + 1) * P],
                        out_offset=None,
                        in_=hs_full[:],
                        in_offset=IndirectOffsetOnAxis(ap=rows_sb[:, q:q + 1], axis=0),
                    ).then_inc(g2_sem, 16)

            gps.wait_ge(v_sem, LM["vE2"][NT - 1])
            gps.dma_start(
                out=out_ext.rearrange("(j p) f -> p j f", p=P),
                in_=out_all[:].rearrange("p (j f) -> p j f", j=NT),
            ).then_inc(d_sem, 16)
            gps.wait_ge(d_sem, D0 + 16 * (2 * (len(XQ) - 1) + 1))

        @block.tensor
        def _(pe):
            pe.wait_ge(d_sem, D0)
            pe.wait_ge(m_sem, 1)
            for j in range(NT):
                if j >= 2:
                    pe.wait_ge(v_sem, LM["dveA_s"][j - 2])
                nc.tensor.matmul(
                    out=psxw[j % 2][:],
                    lhsT=xt_sb[:, j * P:(j + 1) * P],
                    rhs=w1_sb[:, 0:HID], start=True, stop=False,
                ).then_inc(p_sem, 1)
                nc.tensor.matmul(
                    out=psxw[j % 2][:],
                    lhsT=xt_sb[:, NP_CORE + j * P:NP_CORE + (j + 1) * P],
                    rhs=w1_sb[:, HID:2 * HID], start=False, stop=True,
                ).then_inc(p_sem, 1)

            for j in range(NT):
                pe.wait_ge(g1_sem, 16 * CH * (j + 1))
                pe.wait_ge(v_sem, LM["vS1"][j])
                nc.tensor.matmul(
                    out=psc1[j % 2][:], lhsT=ident_sb[:],
                    rhs=xws_all[:, j * HID:(j + 1) * HID],
                    start=True, stop=False,
                ).then_inc(p_sem, 1)
                for ch in range(CH):
                    nc.tensor.matmul(
                        out=psc1[j % 2][:],
                        lhsT=s_sb[j % 3][:, ch * P:(ch + 1) * P],
                        rhs=msg_sb[j % 3][:, ch * P:(ch + 1) * P],
                        start=False, stop=(ch == CH - 1),
                    ).then_inc(p_sem, 1)

            def mm_out(j):
                pe.wait_ge(v_sem, LM["vC2"][j])
                nc.tensor.matmul(
                    out=psout[j % 2][:, 0:2 * LAT],
                    lhsT=st_sb[j % 2][:], rhs=wm_sb[:],
                    start=True, stop=True,
                ).then_inc(p_sem, 1)

            for j in range(NT):
                pe.wait_ge(g2_sem, 16 * CH * (j + 1))
                pe.wait_ge(v_sem, LM["vS2"][j])
                nc.tensor.matmul(
                    out=pss2[j % 2][:],
                    lhsT=hs_all[:, j * HID:(j + 1) * HID],
                    rhs=ident_sb[:],
                    start=True, stop=False,
                ).then_inc(p_sem, 1)
                for ch in range(CH):
                    nc.tensor.matmul(
                        out=pss2[j % 2][:],
                        lhsT=msg_sb[j % 3][:, ch * P:(ch + 1) * P],
                        rhs=s_sb[j % 3][:, ch * P:(ch + 1) * P],
                        start=False, stop=(ch == CH - 1),
                    ).then_inc(p_sem, 1)
                if j >= 1:
                    mm_out(j - 1)
            mm_out(NT - 1)

        @block.vector
        def _(dve):
            for j in range(NT):
                dve.wait_ge(p_sem, LM["peA_mm"][j])
                nc.vector.tensor_scalar(
                    out=xws_all[:, j * HID:(j + 1) * HID], in0=psxw[j % 2][:],
                    scalar1=dinv_sb[:, j:j + 1], scalar2=None,
                    op0=mybir.AluOpType.mult,
                ).then_inc(v_sem, 1)

            def s_build(j, war_lm):
                if war_lm is not None:
                    dve.wait_ge(p_sem, war_lm)
                for ch in range(CH):
                    q = j * CH + ch
                    nc.vector.tensor_tensor(
                        out=s_sb[j % 3][:, ch * P:(ch + 1) * P],
                        in0=colrel_sb[:, q:q + 1].to_broadcast([P, P]),
                        in1=iota_sb[:],
                        op=mybir.AluOpType.is_equal,
                    ).then_inc(v_sem, 1)

            def epi_c1(j):
                dve.wait_ge(p_sem, LM["pe_c1"][j])
                nc.vector.tensor_scalar(
                    out=tmp1_sb[j % 2][:], in0=psc1[j % 2][:],
                    scalar1=dinv_sb[:, j:j + 1], scalar2=None,
                    op0=mybir.AluOpType.mult,
                ).then_inc(v_sem, 1)
                if j >= 2:
                    dve.wait_ge(a_sem, j - 1)
                nc.vector.tensor_tensor(
                    out=tmp2_sb[j % 2][:], in0=tmp1_sb[j % 2][:], in1=b1f_sb[:],
                    op=mybir.AluOpType.add,
                ).then_inc(v_sem, 1)

            s_build(0, None)
            for j in range(1, NT):
                s_build(j, LM["pe_c1"][j - 3] if j >= 3 else None)
                epi_c1(j - 1)
            epi_c1(NT - 1)

            def copy_c2(j):
                dve.wait_ge(p_sem, LM["pe_mm2"][j])
                nc.vector.tensor_copy(out=st_sb[j % 2][:], in_=pss2[j % 2][:]).then_inc(v_sem, 1)

            def epi_c2(j):
                dve.wait_ge(p_sem, LM["pe_O"][j])
                nc.vector.tensor_scalar(
                    out=otmp_sb[j % 2][:], in0=psout[j % 2][:, 0:2 * LAT],
                    scalar1=dinv_sb[:, j:j + 1], scalar2=None,
                    op0=mybir.AluOpType.mult,
                ).then_inc(v_sem, 1)
                nc.vector.tensor_tensor(
                    out=out_all[:, j * 2 * LAT:(j + 1) * 2 * LAT],
                    in0=otmp_sb[j % 2][:], in1=bmf_sb[:],
                    op=mybir.AluOpType.add,
                ).then_inc(v_sem, 1)

            # conv2 stream (mirrors _landmarks sim exactly)
            s_build2 = lambda j, war: s_build(j, war)
            s_build2(0, None)
            s_build2(1, None)
            copy_c2(0)
            for j in range(2, NT):
                s_build2(j, LM["pe_mm2"][j - 3] if j >= 3 else None)
                copy_c2(j - 1)
                epi_c2(j - 2)
            copy_c2(NT - 1)
            epi_c2(NT - 2)
            epi_c2(NT - 1)

        @block.scalar
        def _(act):
            for j in range(NT):
                act.wait_ge(v_sem, LM["vE1"][j])
                nc.scalar.activation(
                    out=hs_all[:, j * HID:(j + 1) * HID],
                    in_=tmp2_sb[j % 2][:],
                    func=mybir.ActivationFunctionType.Relu,
                    scale=dinv_sb[:, j:j + 1],
                ).then_inc(a_sem, 1)

    es.close()
    return nc


# ------------------------------------------------------------------- host --
def _preprocess(x, edge_index, W1, b1, Wmu, bmu, Wlv, blv):
    import ml_dtypes
    bf16 = ml_dtypes.bfloat16

    ei = np.asarray(edge_index)
    srcs = ei[0].astype(np.int64)
    dst = ei[1].astype(np.int64)

    # degree includes the self-loop; self-loop contribution is added on-device
    deg = (np.bincount(dst, minlength=N) + 1).astype(np.float32)
    dinv = np.zeros(NPAD, dtype=np.float32)
    dinv[:N] = 1.0 / np.sqrt(deg)

    order = np.argsort(dst, kind="stable")
    rs = srcs[order].astype(np.int32)
    cs = dst[order]

    tile_of = cs // P
    n_tiles = NPAD // P
    tile_cnt = np.bincount(tile_of, minlength=n_tiles)
    if tile_cnt.max() > CH * P:
        raise OverflowError("tile edge count exceeds static chunk budget")
    tile_start = np.zeros(n_tiles + 1, dtype=np.int64)
    np.cumsum(tile_cnt, out=tile_start[1:])
    rank = np.arange(len(cs)) - tile_start[tile_of]
    pos = tile_of * (CH * P) + rank

    # remap row ids to the quartered xws_full/hs_full layout:
    # quarter q holds rows {core k, local tile in [XQ[q], XQ[q+1])} contiguously
    v_all = np.arange(NPAD, dtype=np.int64)
    k_of = v_all // NP_CORE
    r_of = v_all % NP_CORE
    t_of = r_of // P
    q_of = np.searchsorted(np.asarray(XQ[1:]), t_of, side="right")
    nq = np.diff(np.asarray(XQ)) * P                      # rows per core per quarter
    QR = np.concatenate([[0], np.cumsum(N_CORES * nq)])   # quarter bases
    remap = (QR[q_of] + k_of * nq[q_of] + (r_of - np.asarray(XQ)[q_of] * P)).astype(np.int32)

    rows_pad = np.zeros(n_tiles * CH * P, dtype=np.int32)
    colr_pad = np.full(n_tiles * CH * P, 999.0, dtype=np.float32)
    rows_pad[pos] = remap[rs]
    colr_pad[pos] = (cs - tile_of * P).astype(np.float32)
    rows_pad = rows_pad.reshape(N_CORES, NT * CH, P)
    colr_pad = colr_pad.reshape(N_CORES, NT * CH, P)

    xb = np.asarray(x, dtype=np.float32).astype(bf16)
    x_pad = np.zeros((NPAD, IN_C), dtype=bf16)
    x_pad[:N] = xb
    xT = np.ascontiguousarray(x_pad.T)                        # [IN_C, NPAD]
    xt_cat = np.ascontiguousarray(
        xT.reshape(IN_C, N_CORES, NP_CORE).transpose(1, 0, 2)
    ).reshape(N_CORES * IN_C, NP_CORE)

    w1 = np.asarray(W1, dtype=np.float32).astype(bf16)
    wm = np.concatenate([np.asarray(Wmu), np.asarray(Wlv)], axis=1).astype(np.float32).astype(bf16)
    iota = np.tile(np.arange(P, dtype=np.float32), (P, 1))
    b1f = np.tile(np.asarray(b1, dtype=np.float32)[None, :], (P, 1))
    bmf = np.tile(np.concatenate([np.asarray(bmu), np.asarray(blv)]).astype(np.float32)[None, :], (P, 1))

    dinv_t = dinv.reshape(N_CORES, NT, P).transpose(0, 2, 1)
    rows_t = np.ascontiguousarray(rows_pad.transpose(0, 2, 1))
    colr_t = np.ascontiguousarray(colr_pad.transpose(0, 2, 1))

    cat = {
        "xt": xt_cat,
        "w1": np.tile(w1, (N_CORES, 1)),
        "wmulv": np.tile(wm, (N_CORES, 1)),
        "iota": np.tile(iota, (N_CORES, 1)),
        "b1f": np.tile(b1f, (N_CORES, 1)),
        "bmf": np.tile(bmf, (N_CORES, 1)),
        "dinv": np.ascontiguousarray(dinv_t.reshape(N_CORES * P, NT)),
        "rows": np.ascontiguousarray(rows_t.reshape(N_CORES * P, NT * CH)),
        "colrel": np.ascontiguousarray(colr_t.reshape(N_CORES * P, NT * CH)),
    }
    return cat


def make_in_maps(cat):
    """Split concat inputs back into per-core maps (for test/profiling)."""
    maps = []
    for c in range(N_CORES):
        m = {}
        for k, v in cat.items():
            per = v.shape[0] // N_CORES
            m[k] = np.ascontiguousarray(v[c * per:(c + 1) * per])
        maps.append(m)
    return maps


def _get_runner():
    global _RUNNER, _NC
    if _RUNNER is None:
        import jax
        from jax.sharding import Mesh, PartitionSpec
        from jax.experimental.shard_map import shard_map
        import concourse.mybir as mybir
        from concourse.bass2jax import (
            _bass_exec_p, install_neuronx_cc_hook, partition_id_tensor,
        )

        install_neuronx_cc_hook()
        nc = _build_program()
        _NC = nc

        partition_name = nc.partition_id_tensor.name if nc.partition_id_tensor else None
        in_names, out_names, out_avals = [], [], []
        for alloc in nc.m.functions[0].allocations:
            if not isinstance(alloc, mybir.MemoryLocationSet):
                continue
            name = alloc.memorylocations[0].name
            if alloc.kind == "ExternalInput":
                if name != partition_name:
                    in_names.append(name)
            elif alloc.kind == "ExternalOutput":
                out_names.append(name)
                out_avals.append(jax.core.ShapedArray(
                    tuple(alloc.tensor_shape), mybir.dt.np(alloc.dtype)))

        n_params, n_outs = len(in_names), len(out_names)
        all_in_names = in_names + out_names + ([partition_name] if partition_name else [])
        donate = tuple(range(n_params, n_params + n_outs))

        def _body(*args):
            operands = list(args)
            if partition_name is not None:
                operands.append(partition_id_tensor())
            return tuple(_bass_exec_p.bind(
                *operands,
                out_avals=tuple(out_avals),
                in_names=tuple(all_in_names),
                out_names=tuple(out_names),
                lowering_input_output_aliases=(),
                sim_require_finite=False,
                sim_require_nnan=False,
                nc=nc,
            ))

        devices = jax.devices()[:N_CORES]
        mesh = Mesh(np.asarray(devices), ("core",))
        fn = jax.jit(
            shard_map(
                _body, mesh=mesh,
                in_specs=(PartitionSpec("core"),) * (n_params + n_outs),
                out_specs=(PartitionSpec("core"),) * n_outs,
                check_rep=False,
            ),
            donate_argnums=donate, keep_unused=True,
        )

        def call(cat):
            args = [cat[n] for n in in_names]
            zeros = [np.zeros((N_CORES * a.shape[0], *a.shape[1:]), a.dtype)
                     for a in out_avals]
            outs = fn(*args, *zeros)
            return {n: np.asarray(outs[i]) for i, n in enumerate(out_names)}

        _RUNNER = call
    return _RUNNER


def kernel(x, edge_index, W1, b1, Wmu, bmu, Wlv, blv):
    run = _get_runner()
    cat = _preprocess(x, edge_index, W1, b1, Wmu, bmu, Wlv, blv)
    out = run(cat)["out"]
    mu = out.reshape(NPAD, 2 * LAT)[:N, :LAT]
    logvar = out.reshape(NPAD, 2 * LAT)[:N, LAT:]
    return (np.ascontiguousarray(mu), np.ascontiguousarray(logvar))


if os.environ.get("GCN_NO_WARMUP") != "1" and __name__ != "__main__":
    try:
        _get_runner()
    except Exception:
        _RUNNER = None


# revision 15
# speedup vs baseline: 1.0443x; 1.0063x over previous
"""GCN encoder (relu(A@x@W1+b1) -> A@h@{Wmu,Wlv}+{bmu,blv}) on 8 Trainium2
NeuronCores via Bass (axon-tunneled).

Sharding: nodes split contiguously across 8 cores (6272 padded nodes each,
49 tiles of 128).  Per core: local x@W1 (PE-transpose + bf16 matmul),
AllGather of scaled features, SpMM via indirect-DMA row gathers + on-device
one-hot selection matrices contracted on the PE array, AllGather again,
second SpMM, then the two output matmuls fused as one [128h]x[128h,128f].

Self-contained: needs numpy + concourse (on PYTHONPATH) + jax (axon).
"""
import os
import numpy as np

N = 50000
E = 800000
IN_C, HID, LAT = 256, 128, 64
N_CORES = 8
NP_CORE = 6272              # padded nodes per core (49 tiles)
NPAD = N_CORES * NP_CORE    # 50176
NT = NP_CORE // 128         # 49 tiles per core
CH = 17                     # gather chunks (128 edges) per tile (self-loops excluded)
P = 128

_RUNNER = None
_NC = None
XQ = [0, 25, 49]                    # halved allgathers, issued from sync engine


# -------------------------------------------------------- landmark prepass --
def _landmarks():
    """Simulate per-engine instruction streams; return exact semaphore
    landmark tables.  Must mirror the emission order in _build_program."""
    L = {}
    # --- PE stream ---
    p = 0
    L["peA_mm"] = {}
    for j in range(NT):
        p += 2; L["peA_mm"][j] = p          # after both xw matmuls
    L["pe_c1"] = {}
    for j in range(NT):
        p += CH + 1; L["pe_c1"][j] = p      # identity(self) + CH chunks
    L["pe_mm2"] = {}; L["pe_O"] = {}
    for j in range(NT):
        p += CH + 1; L["pe_mm2"][j] = p
        if j >= 1:
            p += 1; L["pe_O"][j - 1] = p
    p += 1; L["pe_O"][NT - 1] = p
    L["pe_total"] = p

    # --- DVE stream ---
    v = 0
    L["dveA_s"] = {}
    for j in range(NT):
        v += 1; L["dveA_s"][j] = v
    L["vS1"] = {}; L["vE1"] = {}

    def sim_s1(j):
        nonlocal v
        v += CH; L["vS1"][j] = v

    def sim_e1(j):
        nonlocal v
        v += 2; L["vE1"][j] = v

    sim_s1(0)
    for j in range(1, NT):
        sim_s1(j); sim_e1(j - 1)
    sim_e1(NT - 1)

    L["vS2"] = {}; L["vC2"] = {}; L["vE2"] = {}

    def sim_s2(j):
        nonlocal v
        v += CH; L["vS2"][j] = v

    def sim_c2(j):
        nonlocal v
        v += 1; L["vC2"][j] = v

    def sim_e2(j):
        nonlocal v
        v += 2; L["vE2"][j] = v

    sim_s2(0)
    sim_s2(1); sim_c2(0)
    for j in range(2, NT):
        sim_s2(j); sim_c2(j - 1); sim_e2(j - 2)
    sim_c2(NT - 1); sim_e2(NT - 2); sim_e2(NT - 1)
    L["v_total"] = v
    return L


# ----------------------------------------------------------------- device --
def _build_program():
    import concourse.bass as bass
    import concourse.mybir as mybir
    from concourse.bass import IndirectOffsetOnAxis
    from contextlib import ExitStack

    f32, bf16, i32 = mybir.dt.float32, mybir.dt.bfloat16, mybir.dt.int32
    LM = _landmarks()

    nc = bass.Bass()

    xt_in = nc.dram_tensor("xt", [IN_C, NP_CORE], bf16, kind="ExternalInput")
    w1_in = nc.dram_tensor("w1", [IN_C, HID], bf16, kind="ExternalInput")
    wm_in = nc.dram_tensor("wmulv", [HID, 2 * LAT], bf16, kind="ExternalInput")
    iota_in = nc.dram_tensor("iota", [P, P], f32, kind="ExternalInput")
    b1f_in = nc.dram_tensor("b1f", [P, HID], f32, kind="ExternalInput")
    bmf_in = nc.dram_tensor("bmf", [P, 2 * LAT], f32, kind="ExternalInput")
    dinv_in = nc.dram_tensor("dinv", [P, NT], f32, kind="ExternalInput")
    rows_in = nc.dram_tensor("rows", [P, NT * CH], i32, kind="ExternalInput")
    colrel_in = nc.dram_tensor("colrel", [P, NT * CH], f32, kind="ExternalInput")
    out_ext = nc.dram_tensor("out", [NP_CORE, 2 * LAT], f32, kind="ExternalOutput")

    xws_local = nc.dram_tensor("xws_local", [NP_CORE, HID], bf16)
    xws_full = nc.dram_tensor("xws_full", [NPAD, HID], bf16, addr_space="Shared")
    hs_local = nc.dram_tensor("hs_local", [NP_CORE, HID], bf16)
    hs_full = nc.dram_tensor("hs_full", [NPAD, HID], bf16, addr_space="Shared")

    es = ExitStack()
    sb = lambda name, shape, dt: es.enter_context(nc.sbuf_tensor(name, shape, dt))
    xt_sb = sb("xt_sb", [P, 2 * NP_CORE], bf16)
    w1_sb = sb("w1_sb", [P, 2 * HID], bf16)
    wm_sb = sb("wm_sb", [P, 2 * LAT], bf16)
    iota_sb = sb("iota_sb", [P, P], f32)
    b1f_sb = sb("b1f_sb", [P, HID], f32)
    bmf_sb = sb("bmf_sb", [P, 2 * LAT], f32)
    dinv_sb = sb("dinv_sb", [P, NT], f32)
    rows_sb = sb("rows_sb", [P, NT * CH], i32)
    colrel_sb = sb("colrel_sb", [P, NT * CH], f32)
    ident_sb = sb("ident_sb", [P, P], bf16)
    xws_all = sb("xws_all", [P, NT * HID], bf16)
    hs_all = sb("hs_all", [P, NT * HID], bf16)
    out_all = sb("out_all", [P, NT * 2 * LAT], f32)
    msg_sb = [sb(f"msg_sb{i}", [P, CH * P], bf16) for i in range(3)]
    s_sb = [sb(f"s_sb{i}", [P, CH * P], bf16) for i in range(3)]
    st_sb = [sb(f"st_sb{i}", [P, P], bf16) for i in range(2)]
    tmp1_sb = [sb(f"tmp1_sb{i}", [P, P], f32) for i in range(2)]
    tmp2_sb = [sb(f"tmp2_sb{i}", [P, P], f32) for i in range(2)]
    otmp_sb = [sb(f"otmp_sb{i}", [P, 2 * LAT], f32) for i in range(2)]

    psum = lambda name, dt: es.enter_context(nc.psum_tensor(name, [P, P], dt))
    psxw = [psum(f"psxw_{i}", f32) for i in range(2)]
    psc1 = [psum(f"psc1_{i}", f32) for i in range(2)]
    pss2 = psxw    # conv2 reuses f32 banks (phase A / conv1 drained by then)
    psout = psc1

    sem = lambda name: es.enter_context(nc.semaphore(name))
    d_sem = sem("d")
    g1_sem = sem("g1")
    g2_sem = sem("g2")
    p_sem = sem("p")
    v_sem = sem("v")
    a_sem = sem("a")
    m_sem = sem("m")
    c_sem = sem("c")

    D0 = 16 * 9  # 8 gpsimd loads + x load (sync queue, same sem)

    with nc.Block() as block:

        @block.sync
        def _(sync):
            sync.dma_start(
                out=xt_sb[:].rearrange("p (c n) -> p c n", c=2),
                in_=xt_in.rearrange("(c p) n -> p c n", p=P),
            ).then_inc(d_sem, 16)

        @block.gpsimd
        def _(gps):
            gps.dma_start(
                out=w1_sb[:].rearrange("p (c h) -> p c h", c=2),
                in_=w1_in.rearrange("(c p) h -> p c h", p=P),
            ).then_inc(d_sem, 16)
            gps.dma_start(out=wm_sb[:], in_=wm_in[:]).then_inc(d_sem, 16)
            gps.dma_start(out=iota_sb[:], in_=iota_in[:]).then_inc(d_sem, 16)
            gps.dma_start(out=b1f_sb[:], in_=b1f_in[:]).then_inc(d_sem, 16)
            gps.dma_start(out=bmf_sb[:], in_=bmf_in[:]).then_inc(d_sem, 16)
            gps.dma_start(out=dinv_sb[:], in_=dinv_in[:]).then_inc(d_sem, 16)
            gps.dma_start(out=rows_sb[:], in_=rows_in[:]).then_inc(d_sem, 16)
            gps.dma_start(out=colrel_sb[:], in_=colrel_in[:]).then_inc(d_sem, 16)
            gps.memset(ident_sb[:], 0.0)
            gps.affine_select(
                out=ident_sb[:], in_=ident_sb[:],
                compare_op=mybir.AluOpType.not_equal, fill=1.0,
                base=0, pattern=[[-1, P]], channel_multiplier=1,
            ).then_inc(m_sem, 1)

            QR = [N_CORES * P * t for t in XQ]

            def xws_ag(q):
                t0q, t1q = XQ[q], XQ[q + 1]
                gps.wait_ge(v_sem, LM["dveA_s"][t1q - 1])
                gps.dma_start(
                    out=xws_local[t0q * P:t1q * P].rearrange("(j p) h -> p j h", p=P),
                    in_=xws_all[:, t0q * HID:t1q * HID].rearrange("p (j h) -> p j h", j=t1q - t0q),
                ).then_inc(d_sem, 16)
                gps.wait_ge(d_sem, D0 + 16 * (q + 1))
                gps.collective_compute(
                    "AllGather", mybir.AluOpType.bypass,
                    ins=[xws_local[t0q * P:t1q * P]],
                    outs=[xws_full[QR[q]:QR[q + 1]]],
                    replica_groups=[list(range(N_CORES))],
                ).then_inc(c_sem, 1)

            def hs_ag(q):
                t0q, t1q = XQ[q], XQ[q + 1]
                gps.wait_ge(a_sem, t1q)
                gps.dma_start(
                    out=hs_local[t0q * P:t1q * P].rearrange("(j p) h -> p j h", p=P),
                    in_=hs_all[:, t0q * HID:t1q * HID].rearrange("p (j h) -> p j h", j=t1q - t0q),
                ).then_inc(d_sem, 16)
                gps.wait_ge(d_sem, D0 + 32 + 16 * (q + 1))
                gps.collective_compute(
                    "AllGather", mybir.AluOpType.bypass,
                    ins=[hs_local[t0q * P:t1q * P]],
                    outs=[hs_full[QR[q]:QR[q + 1]]],
                    replica_groups=[list(range(N_CORES))],
                ).then_inc(c_sem, 1)

            def c1_gathers(j):
                if j >= 3:
                    gps.wait_ge(p_sem, LM["pe_c1"][j - 3])
                for ch in range(CH):
                    q = j * CH + ch
                    gps.indirect_dma_start(
                        out=msg_sb[j % 3][:, ch * P:(ch + 1) * P],
                        out_offset=None,
                        in_=xws_full[:],
                        in_offset=IndirectOffsetOnAxis(ap=rows_sb[:, q:q + 1], axis=0),
                    ).then_inc(g1_sem, 16)

            def c2_gathers(j):
                if j >= 3:
                    gps.wait_ge(p_sem, LM["pe_mm2"][j - 3])
                for ch in range(CH):
                    q = j * CH + ch
                    gps.indirect_dma_start(
                        out=msg_sb[j % 3][:, ch * P:(ch + 1) * P],
                        out_offset=None,
                        in_=hs_full[:],
                        in_offset=IndirectOffsetOnAxis(ap=rows_sb[:, q:q + 1], axis=0),
                    ).then_inc(g2_sem, 16)

            xws_ag(0)
            xws_ag(1)
            gps.wait_ge(c_sem, 2)
            for j in range(XQ[1]):
                c1_gathers(j)
            hs_ag(0)                  # overlaps remaining conv1 gathers
            for j in range(XQ[1], NT):
                c1_gathers(j)
            hs_ag(1)
            gps.wait_ge(c_sem, 4)
            for j in range(NT):
                c2_gathers(j)
            gps.wait_ge(v_sem, LM["vE2"][NT - 1])
            gps.dma_start(
                out=out_ext.rearrange("(j p) f -> p j f", p=P),
                in_=out_all[:].rearrange("p (j f) -> p j f", j=NT),
            ).then_inc(d_sem, 16)
            gps.wait_ge(d_sem, D0 + 80)

        @block.tensor
        def _(pe):
            pe.wait_ge(d_sem, D0)
            pe.wait_ge(m_sem, 1)
            for j in range(NT):
                if j >= 2:
                    pe.wait_ge(v_sem, LM["dveA_s"][j - 2])
                nc.tensor.matmul(
                    out=psxw[j % 2][:],
                    lhsT=xt_sb[:, j * P:(j + 1) * P],
                    rhs=w1_sb[:, 0:HID], start=True, stop=False,
                ).then_inc(p_sem, 1)
                nc.tensor.matmul(
                    out=psxw[j % 2][:],
                    lhsT=xt_sb[:, NP_CORE + j * P:NP_CORE + (j + 1) * P],
                    rhs=w1_sb[:, HID:2 * HID], start=False, stop=True,
                ).then_inc(p_sem, 1)

            for j in range(NT):
                pe.wait_ge(g1_sem, 16 * CH * (j + 1))
                pe.wait_ge(v_sem, LM["vS1"][j])
                nc.tensor.matmul(
                    out=psc1[j % 2][:], lhsT=ident_sb[:],
                    rhs=xws_all[:, j * HID:(j + 1) * HID],
                    start=True, stop=False,
                ).then_inc(p_sem, 1)
                for ch in range(CH):
                    nc.tensor.matmul(
                        out=psc1[j % 2][:],
                        lhsT=s_sb[j % 3][:, ch * P:(ch + 1) * P],
                        rhs=msg_sb[j % 3][:, ch * P:(ch + 1) * P],
                        start=False, stop=(ch == CH - 1),
                    ).then_inc(p_sem, 1)

            def mm_out(j):
                pe.wait_ge(v_sem, LM["vC2"][j])
                nc.tensor.matmul(
                    out=psout[j % 2][:, 0:2 * LAT],
                    lhsT=st_sb[j % 2][:], rhs=wm_sb[:],
                    start=True, stop=True,
                ).then_inc(p_sem, 1)

            for j in range(NT):
                pe.wait_ge(g2_sem, 16 * CH * (j + 1))
                pe.wait_ge(v_sem, LM["vS2"][j])
                nc.tensor.matmul(
                    out=pss2[j % 2][:],
                    lhsT=hs_all[:, j * HID:(j + 1) * HID],
                    rhs=ident_sb[:],
                    start=True, stop=False,
                ).then_inc(p_sem, 1)
                for ch in range(CH):
                    nc.tensor.matmul(
                        out=pss2[j % 2][:],
                        lhsT=msg_sb[j % 3][:, ch * P:(ch + 1) * P],
                        rhs=s_sb[j % 3][:, ch * P:(ch + 1) * P],
                        start=False, stop=(ch == CH - 1),
                    ).then_inc(p_sem, 1)
                if j >= 1:
                    mm_out(j - 1)
            mm_out(NT - 1)

        @block.vector
        def _(dve):
            for j in range(NT):
                dve.wait_ge(p_sem, LM["peA_mm"][j])
                nc.vector.tensor_scalar(
                    out=xws_all[:, j * HID:(j + 1) * HID], in0=psxw[j % 2][:],
                    scalar1=dinv_sb[:, j:j + 1], scalar2=None,
                    op0=mybir.AluOpType.mult,
                ).then_inc(v_sem, 1)

            def s_build(j, war_lm):
                if war_lm is not None:
                    dve.wait_ge(p_sem, war_lm)
                for ch in range(CH):
                    q = j * CH + ch
                    nc.vector.tensor_tensor(
                        out=s_sb[j % 3][:, ch * P:(ch + 1) * P],
                        in0=colrel_sb[:, q:q + 1].to_broadcast([P, P]),
                        in1=iota_sb[:],
                        op=mybir.AluOpType.is_equal,
                    ).then_inc(v_sem, 1)

            def epi_c1(j):
                dve.wait_ge(p_sem, LM["pe_c1"][j])
                nc.vector.tensor_scalar(
                    out=tmp1_sb[j % 2][:], in0=psc1[j % 2][:],
                    scalar1=dinv_sb[:, j:j + 1], scalar2=None,
                    op0=mybir.AluOpType.mult,
                ).then_inc(v_sem, 1)
                if j >= 2:
                    dve.wait_ge(a_sem, j - 1)
                nc.vector.tensor_tensor(
                    out=tmp2_sb[j % 2][:], in0=tmp1_sb[j % 2][:], in1=b1f_sb[:],
                    op=mybir.AluOpType.add,
                ).then_inc(v_sem, 1)

            s_build(0, None)
            for j in range(1, NT):
                s_build(j, LM["pe_c1"][j - 3] if j >= 3 else None)
                epi_c1(j - 1)
            epi_c1(NT - 1)

            def copy_c2(j):
                dve.wait_ge(p_sem, LM["pe_mm2"][j])
                nc.vector.tensor_copy(out=st_sb[j % 2][:], in_=pss2[j % 2][:]).then_inc(v_sem, 1)

            def epi_c2(j):
                dve.wait_ge(p_sem, LM["pe_O"][j])
                nc.vector.tensor_scalar(
                    out=otmp_sb[j % 2][:], in0=psout[j % 2][:, 0:2 * LAT],
                    scalar1=dinv_sb[:, j:j + 1], scalar2=None,
                    op0=mybir.AluOpType.mult,
                ).then_inc(v_sem, 1)
                nc.vector.tensor_tensor(
                    out=out_all[:, j * 2 * LAT:(j + 1) * 2 * LAT],
                    in0=otmp_sb[j % 2][:], in1=bmf_sb[:],
                    op=mybir.AluOpType.add,
                ).then_inc(v_sem, 1)

            # conv2 stream (mirrors _landmarks sim exactly)
            s_build2 = lambda j, war: s_build(j, war)
            s_build2(0, None)
            s_build2(1, None)
            copy_c2(0)
            for j in range(2, NT):
                s_build2(j, LM["pe_mm2"][j - 3] if j >= 3 else None)
                copy_c2(j - 1)
                epi_c2(j - 2)
            copy_c2(NT - 1)
            epi_c2(NT - 2)
            epi_c2(NT - 1)

        @block.scalar
        def _(act):
            for j in range(NT):
                act.wait_ge(v_sem, LM["vE1"][j])
                nc.scalar.activation(
                    out=hs_all[:, j * HID:(j + 1) * HID],
                    in_=tmp2_sb[j % 2][:],
                    func=mybir.ActivationFunctionType.Relu,
                    scale=dinv_sb[:, j:j + 1],
                ).then_inc(a_sem, 1)

    es.close()
    return nc


# ------------------------------------------------------------------- host --
def _preprocess(x, edge_index, W1, b1, Wmu, bmu, Wlv, blv):
    import ml_dtypes
    bf16 = ml_dtypes.bfloat16

    ei = np.asarray(edge_index)
    srcs = ei[0].astype(np.int64)
    dst = ei[1].astype(np.int64)

    # degree includes the self-loop; self-loop contribution is added on-device
    deg = (np.bincount(dst, minlength=N) + 1).astype(np.float32)
    dinv = np.zeros(NPAD, dtype=np.float32)
    dinv[:N] = 1.0 / np.sqrt(deg)

    order = np.argsort(dst, kind="stable")
    rs = srcs[order].astype(np.int32)
    cs = dst[order]

    tile_of = cs // P
    n_tiles = NPAD // P
    tile_cnt = np.bincount(tile_of, minlength=n_tiles)
    if tile_cnt.max() > CH * P:
        raise OverflowError("tile edge count exceeds static chunk budget")
    tile_start = np.zeros(n_tiles + 1, dtype=np.int64)
    np.cumsum(tile_cnt, out=tile_start[1:])
    rank = np.arange(len(cs)) - tile_start[tile_of]
    pos = tile_of * (CH * P) + rank

    # remap row ids to the quartered xws_full/hs_full layout:
    # quarter q holds rows {core k, local tile in [XQ[q], XQ[q+1])} contiguously
    v_all = np.arange(NPAD, dtype=np.int64)
    k_of = v_all // NP_CORE
    r_of = v_all % NP_CORE
    t_of = r_of // P
    q_of = np.searchsorted(np.asarray(XQ[1:]), t_of, side="right")
    nq = np.diff(np.asarray(XQ)) * P                      # rows per core per quarter
    QR = np.concatenate([[0], np.cumsum(N_CORES * nq)])   # quarter bases
    remap = (QR[q_of] + k_of * nq[q_of] + (r_of - np.asarray(XQ)[q_of] * P)).astype(np.int32)

    rows_pad = np.zeros(n_tiles * CH * P, dtype=np.int32)
    colr_pad = np.full(n_tiles * CH * P, 999.0, dtype=np.float32)
    rows_pad[pos] = remap[rs]
    colr_pad[pos] = (cs - tile_of * P).astype(np.float32)
    rows_pad = rows_pad.reshape(N_CORES, NT * CH, P)
    colr_pad = colr_pad.reshape(N_CORES, NT * CH, P)

    xb = np.asarray(x, dtype=np.float32).astype(bf16)
    x_pad = np.zeros((NPAD, IN_C), dtype=bf16)
    x_pad[:N] = xb
    xT = np.ascontiguousarray(x_pad.T)                        # [IN_C, NPAD]
    xt_cat = np.ascontiguousarray(
        xT.reshape(IN_C, N_CORES, NP_CORE).transpose(1, 0, 2)
    ).reshape(N_CORES * IN_C, NP_CORE)

    w1 = np.asarray(W1, dtype=np.float32).astype(bf16)
    wm = np.concatenate([np.asarray(Wmu), np.asarray(Wlv)], axis=1).astype(np.float32).astype(bf16)
    iota = np.tile(np.arange(P, dtype=np.float32), (P, 1))
    b1f = np.tile(np.asarray(b1, dtype=np.float32)[None, :], (P, 1))
    bmf = np.tile(np.concatenate([np.asarray(bmu), np.asarray(blv)]).astype(np.float32)[None, :], (P, 1))

    dinv_t = dinv.reshape(N_CORES, NT, P).transpose(0, 2, 1)
    rows_t = np.ascontiguousarray(rows_pad.transpose(0, 2, 1))
    colr_t = np.ascontiguousarray(colr_pad.transpose(0, 2, 1))

    cat = {
        "xt": xt_cat,
        "w1": np.tile(w1, (N_CORES, 1)),
        "wmulv": np.tile(wm, (N_CORES, 1)),
        "iota": np.tile(iota, (N_CORES, 1)),
        "b1f": np.tile(b1f, (N_CORES, 1)),
        "bmf": np.tile(bmf, (N_CORES, 1)),
        "dinv": np.ascontiguousarray(dinv_t.reshape(N_CORES * P, NT)),
        "rows": np.ascontiguousarray(rows_t.reshape(N_CORES * P, NT * CH)),
        "colrel": np.ascontiguousarray(colr_t.reshape(N_CORES * P, NT * CH)),
    }
    return cat


def make_in_maps(cat):
    """Split concat inputs back into per-core maps (for test/profiling)."""
    maps = []
    for c in range(N_CORES):
        m = {}
        for k, v in cat.items():
            per = v.shape[0] // N_CORES
            m[k] = np.ascontiguousarray(v[c * per:(c + 1) * per])
        maps.append(m)
    return maps


def _get_runner():
    global _RUNNER, _NC
    if _RUNNER is None:
        import jax
        from jax.sharding import Mesh, PartitionSpec
        from jax.experimental.shard_map import shard_map
        import concourse.mybir as mybir
        from concourse.bass2jax import (
            _bass_exec_p, install_neuronx_cc_hook, partition_id_tensor,
        )

        install_neuronx_cc_hook()
        nc = _build_program()
        _NC = nc

        partition_name = nc.partition_id_tensor.name if nc.partition_id_tensor else None
        in_names, out_names, out_avals = [], [], []
        for alloc in nc.m.functions[0].allocations:
            if not isinstance(alloc, mybir.MemoryLocationSet):
                continue
            name = alloc.memorylocations[0].name
            if alloc.kind == "ExternalInput":
                if name != partition_name:
                    in_names.append(name)
            elif alloc.kind == "ExternalOutput":
                out_names.append(name)
                out_avals.append(jax.core.ShapedArray(
                    tuple(alloc.tensor_shape), mybir.dt.np(alloc.dtype)))

        n_params, n_outs = len(in_names), len(out_names)
        all_in_names = in_names + out_names + ([partition_name] if partition_name else [])
        donate = tuple(range(n_params, n_params + n_outs))

        def _body(*args):
            operands = list(args)
            if partition_name is not None:
                operands.append(partition_id_tensor())
            return tuple(_bass_exec_p.bind(
                *operands,
                out_avals=tuple(out_avals),
                in_names=tuple(all_in_names),
                out_names=tuple(out_names),
                lowering_input_output_aliases=(),
                sim_require_finite=False,
                sim_require_nnan=False,
                nc=nc,
            ))

        devices = jax.devices()[:N_CORES]
        mesh = Mesh(np.asarray(devices), ("core",))
        fn = jax.jit(
            shard_map(
                _body, mesh=mesh,
                in_specs=(PartitionSpec("core"),) * (n_params + n_outs),
                out_specs=(PartitionSpec("core"),) * n_outs,
                check_rep=False,
            ),
            donate_argnums=donate, keep_unused=True,
        )

        def call(cat):
            args = [cat[n] for n in in_names]
            zeros = [np.zeros((N_CORES * a.shape[0], *a.shape[1:]), a.dtype)
                     for a in out_avals]
            outs = fn(*args, *zeros)
            return {n: np.asarray(outs[i]) for i, n in enumerate(out_names)}

        _RUNNER = call
    return _RUNNER


def kernel(x, edge_index, W1, b1, Wmu, bmu, Wlv, blv):
    run = _get_runner()
    cat = _preprocess(x, edge_index, W1, b1, Wmu, bmu, Wlv, blv)
    out = run(cat)["out"]
    mu = out.reshape(NPAD, 2 * LAT)[:N, :LAT]
    logvar = out.reshape(NPAD, 2 * LAT)[:N, LAT:]
    return (np.ascontiguousarray(mu), np.ascontiguousarray(logvar))


if os.environ.get("GCN_NO_WARMUP") != "1" and __name__ != "__main__":
    try:
        _get_runner()
    except Exception:
        _RUNNER = None
